# revision 1
# baseline (speedup 1.0000x reference)
"""Trainium2 Bass kernel for the AttentionLayer problem.

Computation (per batch b):
    q = query[b] @ Wq + bq            [S, A]
    v = value[b] @ Wv + bv            [S, A]
    scores = q @ v.T                  [S, S]
    attn = softmax(scores, -1)
    out[b] = attn @ v                 [S, A]

with B=4, S=2048, HIDDEN=A=1024, fp32.

Sharding: 8 cores = (batch b in 0..3) x (query-row half h in 0..1).
Each core handles 1024 query rows of one batch and computes the full
v/vT projection for its batch (duplicated across the pair of cores
sharing a batch; avoids any collectives).

Device-side layout strategy (per core):
  - Inputs arrive s-major [s, hidden]; all matmuls contract over the
    partition dim, so activations are transposed on-device via PE
    identity-matmul transposes.
  - Matmuls run in float32r (4x faster than fp32 on the PE); operands
    are rounded to f32r by the PSUM->SBUF copy-out of the producing
    stage (or by a dedicated conversion copy for DMA'd weights).
  - scores[q,k] tiles: lhsT = qT [a, q-block], rhs = vT [a, k].
  - softmax: no max-subtraction needed (|scores| <~ 60 << fp32 exp
    range); ACT Exp with fused row-sum accumulation, DVE reciprocal,
    normalization folded into the context copy-out.
  - context[q,d]: lhsT = attnT (PE-transposed attn), rhs = v [k, d];
    the softmax normalization (1/rowsum) is applied by the context
    PSUM->SBUF copy-out (per-partition tensor_scalar on DVE).

Measured on trn2 (8 cores): ~255 us/exec (repeat-slope method), max
relative error 2.8e-3 vs the fp32 jax reference (f32r matmul rounding,
~1.6e-4 per matmul, amplified through the softmax).
"""

import sys

if "/opt/trn_rl_repo" not in sys.path:
    sys.path.insert(0, "/opt/trn_rl_repo")

import numpy as np

import concourse.bass as bass
import concourse.mybir as mybir
from concourse import bacc, tile
from concourse.bass_utils import run_bass_kernel_spmd
from concourse.masks import make_identity

F32 = mybir.dt.float32
F32R = mybir.dt.float32r

B, S, H, A = 4, 2048, 1024, 1024
SQ = S // 2  # query rows per core
P = 128
N_CORES = 8

# matmul dtype for PE: float32r (1 cycle/row for N>=256) vs float32 (4 cycles/row)
USE_F32R = True
MM_DT = F32R if USE_F32R else F32
# Optional: context path (attn weights + v) in fp16 — normalize softmax
# before the context matmul, fp16 PE transposes/matmuls. Equal accuracy
# (2.84e-3 vs 2.82e-3) but measured slightly slower end-to-end than the
# f32r context path, so off by default.
CTX_FP16 = False
F16 = mybir.dt.float16
CTX_DT = F16 if CTX_FP16 else MM_DT
CTX_N = 512

Exp = mybir.ActivationFunctionType.Exp
Identity = mybir.ActivationFunctionType.Identity


def build(repeat: int = 1, repeat_a: int = 1):
    nc = bacc.Bacc(None, target_bir_lowering=False, debug=False)

    xq = nc.dram_tensor("xq", [SQ, H], F32, kind="ExternalInput")
    xv = nc.dram_tensor("xv", [S, H], F32, kind="ExternalInput")
    wq = nc.dram_tensor("wq", [P, H // P, A], F32, kind="ExternalInput")
    wv = nc.dram_tensor("wv", [P, H // P, A], F32, kind="ExternalInput")
    bq = nc.dram_tensor("bq", [P, A // P], F32, kind="ExternalInput")
    bv = nc.dram_tensor("bv", [P, A // P], F32, kind="ExternalInput")
    out = nc.dram_tensor("out", [SQ, A], F32, kind="ExternalOutput")

    xq_t = xq.rearrange("(o p) f -> o p f", p=P)  # [8, 128, 1024]
    xv_t = xv.rearrange("(o p) f -> o p f", p=P)  # [16, 128, 1024]
    out_t = out.rearrange("(o p) f -> o p f", p=P)  # [8, 128, 1024]

    KO = H // P  # 8 contraction chunks of 128
    AO = A // P  # 8 a-tiles
    SO = S // P  # 16 key tiles
    QO = SQ // P  # 8 query tiles per core

    with tile.TileContext(nc) as tc:
        with (
            tc.tile_pool(name="persist", bufs=1) as persist,
            tc.tile_pool(name="psum", bufs=1, space="PSUM") as psum,
        ):
            # ---- persistent tiles ----
            ident = persist.tile([P, P], F32, name="ident")
            make_identity(nc, ident[:])
            identr = persist.tile([P, P], MM_DT, name="identr")
            nc.vector.tensor_copy(identr[:], ident[:])
            if CTX_FP16:
                identh = persist.tile([P, P], CTX_DT, name="identh")
                nc.vector.tensor_copy(identh[:], ident[:])
            else:
                identh = identr
            bq_sb = persist.tile([P, A // P], F32, name="bq_sb")
            nc.sync.dma_start(bq_sb[:], bq[:])
            bv_sb = persist.tile([P, A // P], F32, name="bv_sb")
            nc.sync.dma_start(bv_sb[:], bv[:])

            # vT [a, s] (f32r): scores rhs.   64KB/part
            vT = persist.tile([P, AO, S], MM_DT, name="vT", tag="vT")
            # qT [a, sq] (f32r): scores lhsT. 32KB/part
            qT = persist.tile([P, AO, SQ], MM_DT, name="qT", tag="qT")
            for _rep in range(repeat):
                # valueT [h, s] (f32r) then v [s, a] (f32r) share one 64KB slot
                valueT = persist.tile([P, KO, S], MM_DT, name="valueT", tag="vv")
                proj = tc.alloc_tile_pool(name="proj", bufs=1)

                def load_weights_f32r(w_dram, name):
                    # DMA f32 chunks via a small raw slot, convert to f32r
                    wr = proj.tile([P, KO, A], MM_DT, name=name, tag="W")
                    for ko in range(KO):
                        for hf in range(2):
                            rawW = proj.tile(
                                [P, 512], F32, name=f"rawW_{name}_{ko}_{hf}", tag="raw", bufs=2
                            )
                            nc.sync.dma_start(rawW[:], w_dram[:, ko, hf * 512 : (hf + 1) * 512])
                            nc.vector.tensor_copy(
                                wr[:, ko, hf * 512 : (hf + 1) * 512], rawW[:]
                            )
                    return wr

                def transpose_group(dst_ap, src_blocks, tag):
                    """PE-transpose up to 4 [128,128] f32 blocks into one PSUM
                    tile, then one copy-out (converting to dst dtype)."""
                    n = len(src_blocks)
                    tpp = psum.tile([P, n * P], F32, name=f"tp_{tag}", tag="tp")
                    for j, blk in enumerate(src_blocks):
                        nc.tensor.transpose(tpp[:, j * P : (j + 1) * P], blk, ident[:])
                    nc.scalar.copy(dst_ap, tpp[:])

                # ================= phase V: value projections =================
                wv_sb = load_weights_f32r(wv, "wv_sb")

                # build valueT (transpose value tiles)
                for so in range(SO):
                    for hg in range(2):
                        raw = proj.tile([P, 512], F32, name=f"rawv_{so}_{hg}", tag="raw", bufs=2)
                        nc.sync.dma_start(raw[:], xv_t[so][:, hg * 512 : (hg + 1) * 512])
                        # dst: valueT[:, hg*4:(hg+1)*4, so*128:+128] viewed [128, 4, 128]
                        dst = valueT[:, hg * 4 : (hg + 1) * 4, so * P : (so + 1) * P]
                        tpp = psum.tile([P, 4 * P], F32, name=f"tpv_{so}_{hg}", tag="tp")
                        for j in range(4):
                            nc.tensor.transpose(
                                tpp[:, j * P : (j + 1) * P],
                                raw[:, j * P : (j + 1) * P],
                                ident[:],
                            )
                        nc.scalar.copy(dst, tpp.rearrange("p (a b) -> p a b", a=4))

                # vT proj: out [a, s]; psum [128, 1024] per (ai, so2)
                for ai in range(AO):
                    for so2 in range(2):  # s in chunks of 1024
                        pp = psum.tile([P, 1024], F32, name=f"pvT_{ai}_{so2}", tag="quad", bufs=2)
                        for k in range(KO):
                            for sc in range(2):  # 512-wide matmuls
                                nc.tensor.matmul(
                                    pp[:, sc * 512 : (sc + 1) * 512],
                                    wv_sb[:, k, ai * P : (ai + 1) * P],
                                    valueT[:, k, so2 * 1024 + sc * 512 : so2 * 1024 + (sc + 1) * 512],
                                    start=(k == 0),
                                    stop=(k == KO - 1),
                                )
                        nc.scalar.activation(
                            vT[:, ai, so2 * 1024 : (so2 + 1) * 1024],
                            pp[:],
                            Identity,
                            bias=bv_sb[:, ai : ai + 1],
                        )

                # v [s, a] (f32r), same slot as valueT
                v_sb = persist.tile([P, SO, A], CTX_DT, name="v_sb", tag="vv")
                for so in range(SO):
                    for ag in range(2):
                        tpp = psum.tile([P, 4 * P], MM_DT, name=f"tpvv_{so}_{ag}", tag="tp")
                        for j in range(4):
                            ai = ag * 4 + j
                            nc.tensor.transpose(
                                tpp[:, j * P : (j + 1) * P],
                                vT[:, ai, so * P : (so + 1) * P],
                                identr[:],
                            )
                        nc.scalar.copy(
                            v_sb[:, so, ag * 512 : (ag + 1) * 512], tpp[:]
                        )

                # ================= phase Q: query projection =================
                wq_sb = load_weights_f32r(wq, "wq_sb")
                for qc in range(4):  # 256-column chunks of queryT
                    qTc = proj.tile([P, KO, 256], MM_DT, name=f"qTc_{qc}", tag="qTc", bufs=1)
                    for r in range(2):
                        so = qc * 2 + r
                        for hg in range(2):
                            raw = proj.tile(
                                [P, 512], F32, name=f"rawq_{so}_{hg}", tag="raw", bufs=2
                            )
                            nc.sync.dma_start(raw[:], xq_t[so][:, hg * 512 : (hg + 1) * 512])
                            dst = qTc[:, hg * 4 : (hg + 1) * 4, r * P : (r + 1) * P]
                            tpp = psum.tile([P, 4 * P], F32, name=f"tpq_{so}_{hg}", tag="tp")
                            for j in range(4):
                                nc.tensor.transpose(
                                    tpp[:, j * P : (j + 1) * P],
                                    raw[:, j * P : (j + 1) * P],
                                    ident[:],
                                )
                            nc.scalar.copy(dst, tpp.rearrange("p (a b) -> p a b", a=4))
                    for ai in range(AO):
                        pp = psum.tile([P, 1024], F32, name=f"pq_{qc}_{ai}", tag="quad", bufs=2)
                        for k in range(KO):
                            nc.tensor.matmul(
                                pp[:, :256],
                                wq_sb[:, k, ai * P : (ai + 1) * P],
                                qTc[:, k, :],
                                start=(k == 0),
                                stop=(k == KO - 1),
                            )
                        nc.scalar.activation(
                            qT[:, ai, qc * 256 : (qc + 1) * 256],
                            pp[:, :256],
                            Identity,
                            bias=bq_sb[:, ai : ai + 1],
                        )

                proj.release()
                attnp = tc.alloc_tile_pool(name="attnp", bufs=1)

                # ================= phase A: attention =================
                for qi in range(QO * repeat_a):
                    qi = qi % QO
                    # scores [128 q, 2048 k] in two psum halves
                    halves = []
                    for hf in range(2):
                        ph = psum.tile([P, 1024], F32, name=f"ps_{qi}_{hf}", tag="quad", bufs=2)
                        for nck in range(2):
                            for k in range(AO):
                                nc.tensor.matmul(
                                    ph[:, nck * 512 : (nck + 1) * 512],
                                    qT[:, k, qi * P : (qi + 1) * P],
                                    vT[:, k, hf * 1024 + nck * 512 : hf * 1024 + (nck + 1) * 512],
                                    start=(k == 0),
                                    stop=(k == AO - 1),
                                )
                        halves.append(ph)

                    attn = attnp.tile([P, S], F32, name=f"attn_{qi}", tag="attn", bufs=3)
                    s0 = attnp.tile([P, 1], F32, name=f"s0_{qi}", tag="s0", bufs=2)
                    s1 = attnp.tile([P, 1], F32, name=f"s1_{qi}", tag="s1", bufs=2)
                    nc.scalar.activation(attn[:, :1024], halves[0][:], Exp, accum_out=s0[:])
                    nc.scalar.activation(attn[:, 1024:], halves[1][:], Exp, accum_out=s1[:])
                    recip = attnp.tile([P, 1], F32, name=f"rc_{qi}", tag="rc", bufs=2)
                    nc.vector.tensor_add(recip[:], s0[:], s1[:])
                    nc.vector.reciprocal(recip[:], recip[:])

                    if CTX_FP16:
                        # normalize softmax now (per-partition recip) and
                        # cast to fp16 for the context matmul
                        attn_n = attnp.tile([P, S], CTX_DT, name=f"attn_n_{qi}", tag="attn_n", bufs=2)
                        for hf in range(2):
                            nc.vector.tensor_scalar_mul(
                                attn_n[:, hf * 1024 : (hf + 1) * 1024],
                                attn[:, hf * 1024 : (hf + 1) * 1024],
                                recip[:],
                            )
                        tp_src, tp_ident = attn_n, identh
                    else:
                        tp_src, tp_ident = attn, ident

                    attnT = attnp.tile([P, SO, P], CTX_DT, name=f"attnT_{qi}", tag="attnT", bufs=2)
                    for kg in range(4):
                        tpp = psum.tile([P, 4 * P], CTX_DT if CTX_FP16 else F32, name=f"tpa_{qi}_{kg}", tag="tp")
                        for j in range(4):
                            kb = kg * 4 + j
                            nc.tensor.transpose(
                                tpp[:, j * P : (j + 1) * P],
                                tp_src[:, kb * P : (kb + 1) * P],
                                tp_ident[:],
                            )
                        # ACT for these copy-outs: matches the measured-best
                        # schedule (DVE handles recip/normalize in this phase)
                        nc.scalar.copy(
                            attnT[:, kg * 4 : (kg + 1) * 4, :],
                            tpp.rearrange("p (a b) -> p a b", a=4),
                        )

                    ctx = psum.tile([P, A], F32, name=f"ctx_{qi}", tag="ctx", bufs=1)
                    for dc in range(A // CTX_N):
                        for kb in range(SO):
                            nc.tensor.matmul(
                                ctx[:, dc * CTX_N : (dc + 1) * CTX_N],
                                attnT[:, kb, :],
                                v_sb[:, kb, dc * CTX_N : (dc + 1) * CTX_N],
                                start=(kb == 0),
                                stop=(kb == SO - 1),
                            )
                    outt = attnp.tile([P, A], F32, name=f"out_{qi}", tag="outc", bufs=1)
                    if CTX_FP16:
                        nc.scalar.copy(outt[:], ctx[:])
                    else:
                        nc.vector.tensor_scalar_mul(outt[:], ctx[:], recip[:])
                    nc.sync.dma_start(out_t[qi], outt[:])

                attnp.release()

    nc.compile()
    return nc


_NC_CACHE = {}


def _get_nc():
    if "nc" not in _NC_CACHE:
        _NC_CACHE["nc"] = build()
    return _NC_CACHE["nc"]


def kernel(**inputs):
    query = np.ascontiguousarray(np.asarray(inputs["query"], dtype=np.float32))
    value = np.ascontiguousarray(np.asarray(inputs["value"], dtype=np.float32))
    Wq = np.asarray(inputs["Wq"], dtype=np.float32)
    Wv = np.asarray(inputs["Wv"], dtype=np.float32)
    bqv = np.asarray(inputs["bq"], dtype=np.float32)
    bvv = np.asarray(inputs["bv"], dtype=np.float32)

    # weight pre-tiling (pure layout): [H, A] -> [128, H//128, A]
    wq_t = np.ascontiguousarray(Wq.reshape(H // P, P, A).transpose(1, 0, 2))
    wv_t = np.ascontiguousarray(Wv.reshape(H // P, P, A).transpose(1, 0, 2))
    bq_t = np.ascontiguousarray(bqv.reshape(A // P, P).T)
    bv_t = np.ascontiguousarray(bvv.reshape(A // P, P).T)

    nc = _get_nc()
    in_maps = []
    for c in range(N_CORES):
        b, h = c // 2, c % 2
        in_maps.append(
            {
                "xq": np.ascontiguousarray(query[b, h * SQ : (h + 1) * SQ, :]),
                "xv": value[b],
                "wq": wq_t,
                "wv": wv_t,
                "bq": bq_t,
                "bv": bv_t,
            }
        )
    res = run_bass_kernel_spmd(nc, in_maps, core_ids=list(range(N_CORES)))
    out = np.empty((B, S, A), np.float32)
    for c in range(N_CORES):
        b, h = c // 2, c % 2
        out[b, h * SQ : (h + 1) * SQ, :] = res.results[c]["out"]
    return out



# revision 2
# speedup vs baseline: 6.4484x; 6.4484x over previous
"""Trainium2 Bass kernel for the AttentionLayer problem.

Computation (per batch b):
    q = query[b] @ Wq + bq            [S, A]
    v = value[b] @ Wv + bv            [S, A]
    scores = q @ v.T                  [S, S]
    attn = softmax(scores, -1)
    out[b] = attn @ v                 [S, A]

with B=4, S=2048, HIDDEN=A=1024, fp32 reference; B*S*S*A dominates.

Sharding: 8 cores = (batch b in 0..3) x (query-row half h in 0..1).
Each core handles 1024 query rows of one batch and computes the full
v projection for its batch (duplicated across the pair of cores
sharing a batch; avoids collectives).

Key design points vs the previous f32r version (242 us measured):
  - All matmul operands are fp16 (same 1 cycle/row PE throughput as
    f32r, but fp32-accumulated in PSUM). Host converts inputs to fp16,
    halving HBM traffic.
  - ZERO PE transposes: query/value are loaded pre-transposed via the
    DMA XBAR (dma_start_transpose, 2-byte dtypes), v (s-major) is
    produced from vT by an SBUF->SBUF DMA transpose, and attn^T for
    the context matmul by a per-q-tile SBUF->SBUF DMA transpose.
    The old kernel spent ~107k PE cycles (~45 us) on identity-matmul
    transposes; these now ride the otherwise idle DMA engines.
  - Softmax is row-max-stabilized (DVE negated max reduce feeds the
    ACT Exp bias) so exp() fits fp16; 1/rowsum is folded into the
    context PSUM->SBUF copy-out (DVE per-partition tensor_scalar).
  - Attention is software-pipelined: PE order sc(0), sc(1), cx(0),
    sc(2), cx(1), ... so the max/exp/transpose latency of tile i hides
    under the score matmul of tile i+1. PSUM: sc [128,2048] bufs=1
    (4 banks) + cx [128,1024] bufs=2 (4 banks).

PE work per core: qproj 65536 + vproj 131072 + scores 131072 +
context 131072 = 458752 cycles ~= 191 us at 2.4 GHz peak.
"""

import sys

if "/opt/trn_rl_repo" not in sys.path:
    sys.path.insert(0, "/opt/trn_rl_repo")

import numpy as np

import concourse.bass as bass
import concourse.mybir as mybir
from concourse import bacc, tile
from concourse.bass_utils import run_bass_kernel_spmd

F32 = mybir.dt.float32
F16 = mybir.dt.float16

B, S, H, A = 4, 2048, 1024, 1024
SQ = S // 2  # query rows per core
P = 128
N_CORES = 8
KO = H // P  # 8 contraction chunks of 128
AO = A // P  # 8 a-tiles
SO = S // P  # 16 key tiles
QO = SQ // P  # 8 query tiles per core

Exp = mybir.ActivationFunctionType.Exp
Identity = mybir.ActivationFunctionType.Identity
AxX = mybir.AxisListType.X
MaxOp = mybir.AluOpType.max


def build(repeat: int = 1):
    nc = bacc.Bacc(None, target_bir_lowering=False, debug=False)

    xq = nc.dram_tensor("xq", [SQ, H], F16, kind="ExternalInput")
    xv = nc.dram_tensor("xv", [S, H], F16, kind="ExternalInput")
    wq = nc.dram_tensor("wq", [P, KO, A], F16, kind="ExternalInput")
    wv = nc.dram_tensor("wv", [P, KO, A], F16, kind="ExternalInput")
    bq = nc.dram_tensor("bq", [P, AO], F32, kind="ExternalInput")
    bv = nc.dram_tensor("bv", [P, AO], F32, kind="ExternalInput")
    out = nc.dram_tensor("out", [SQ, A], F32, kind="ExternalOutput")
    out_t = out.rearrange("(o p) f -> o p f", p=P)  # [8, 128, 1024]

    with tile.TileContext(nc) as tc:
        with tc.tile_pool(name="pers", bufs=1) as pers:
            bq_sb = pers.tile([P, AO], F32, name="bq_sb")
            nc.sync.dma_start(bq_sb[:], bq[:])
            bv_sb = pers.tile([P, AO], F32, name="bv_sb")
            nc.sync.dma_start(bv_sb[:], bv[:])

            # persistent activations (a-major / s-major), fp16
            qT = pers.tile([P, AO, SQ], F16, name="qT", tag="qT")  # 16KB/part
            vT = pers.tile([P, AO, S], F16, name="vT", tag="vT")  # 32KB
            v_sb = pers.tile([P, SO, A], F16, name="v_sb", tag="v")  # 32KB

            for _rep in range(repeat):
                proj = tc.alloc_tile_pool(name="proj", bufs=1)
                psp = tc.alloc_tile_pool(name="psp", bufs=1, space="PSUM")

                # ---- input loads (queryT first: q path starts sooner) ----
                wq_sb = proj.tile([P, KO, A], F16, name="wq_sb", tag="wq")
                nc.sync.dma_start(wq_sb[:], wq[:])
                queryT = proj.tile([P, KO, SQ], F16, name="queryT", tag="qry")
                for c in range(2):
                    nc.sync.dma_start_transpose(
                        queryT[:, :, c * 512 : (c + 1) * 512],
                        xq[c * 512 : (c + 1) * 512, :],
                    )
                wv_sb = proj.tile([P, KO, A], F16, name="wv_sb", tag="wv")
                nc.sync.dma_start(wv_sb[:], wv[:])
                valueT = proj.tile([P, KO, S], F16, name="valueT", tag="val")
                for c in range(4):
                    nc.sync.dma_start_transpose(
                        valueT[:, :, c * 512 : (c + 1) * 512],
                        xv[c * 512 : (c + 1) * 512, :],
                    )

                # ---- q projection: qT[a, sq] ----
                for ao in range(AO):
                    pp = psp.tile([P, S], F32, name=f"pq_{ao}", tag="pp", bufs=2)
                    for k in range(KO):
                        for c2 in range(2):
                            nc.tensor.matmul(
                                pp[:, c2 * 512 : (c2 + 1) * 512],
                                wq_sb[:, k, ao * P : (ao + 1) * P],
                                queryT[:, k, c2 * 512 : (c2 + 1) * 512],
                                start=(k == 0),
                                stop=(k == KO - 1),
                            )
                    nc.scalar.activation(
                        qT[:, ao, :], pp[:, :SQ], Identity, bias=bq_sb[:, ao : ao + 1]
                    )

                # ---- v projection: vT[a, s]; v via DMA transpose ----
                for ao in range(AO):
                    pp = psp.tile([P, S], F32, name=f"pv_{ao}", tag="pp", bufs=2)
                    for k in range(KO):
                        for c4 in range(4):
                            nc.tensor.matmul(
                                pp[:, c4 * 512 : (c4 + 1) * 512],
                                wv_sb[:, k, ao * P : (ao + 1) * P],
                                valueT[:, k, c4 * 512 : (c4 + 1) * 512],
                                start=(k == 0),
                                stop=(k == KO - 1),
                            )
                    nc.scalar.activation(
                        vT[:, ao, :], pp[:], Identity, bias=bv_sb[:, ao : ao + 1]
                    )
                    nc.sync.dma_start_transpose(
                        v_sb[:, :, ao * P : (ao + 1) * P], vT[:, ao, :]
                    )

                proj.release()
                psp.release()
                ap = tc.alloc_tile_pool(name="ap", bufs=1)
                psa = tc.alloc_tile_pool(name="psa", bufs=1, space="PSUM")

                # ---- attention, software-pipelined over q-tiles ----
                def scores(qi):
                    sc = psa.tile([P, S], F32, name=f"sc_{qi}", tag="sc", bufs=1)
                    for ach in range(AO):
                        for c4 in range(4):
                            nc.tensor.matmul(
                                sc[:, c4 * 512 : (c4 + 1) * 512],
                                qT[:, ach, qi * P : (qi + 1) * P],
                                vT[:, ach, c4 * 512 : (c4 + 1) * 512],
                                start=(ach == 0),
                                stop=(ach == AO - 1),
                            )
                    negmax = ap.tile([P, 1], F32, name=f"nm_{qi}", tag="nm", bufs=2)
                    nc.vector.tensor_reduce(
                        negmax[:], sc[:], AxX, MaxOp, negate=True
                    )
                    attn = ap.tile([P, S], F16, name=f"at_{qi}", tag="attn", bufs=2)
                    s0 = ap.tile([P, 1], F32, name=f"s0_{qi}", tag="s0", bufs=2)
                    s1 = ap.tile([P, 1], F32, name=f"s1_{qi}", tag="s1", bufs=2)
                    nc.scalar.activation(
                        attn[:, :1024], sc[:, :1024], Exp,
                        bias=negmax[:], accum_out=s0[:],
                    )
                    nc.scalar.activation(
                        attn[:, 1024:], sc[:, 1024:], Exp,
                        bias=negmax[:], accum_out=s1[:],
                    )
                    recip = ap.tile([P, 1], F32, name=f"rc_{qi}", tag="rc", bufs=2)
                    nc.vector.tensor_add(recip[:], s0[:], s1[:])
                    nc.vector.reciprocal(recip[:], recip[:])
                    attnT = ap.tile([P, SO, P], F16, name=f"aT_{qi}", tag="aT", bufs=2)
                    nc.sync.dma_start_transpose(attnT[:], attn[:])
                    return attnT, recip

                def context(qi, attnT, recip):
                    cx = psa.tile([P, A], F32, name=f"cx_{qi}", tag="cx", bufs=2)
                    for kb in range(SO):
                        for c2 in range(2):
                            nc.tensor.matmul(
                                cx[:, c2 * 512 : (c2 + 1) * 512],
                                attnT[:, kb, :],
                                v_sb[:, kb, c2 * 512 : (c2 + 1) * 512],
                                start=(kb == 0),
                                stop=(kb == SO - 1),
                            )
                    outt = ap.tile([P, A], F32, name=f"ot_{qi}", tag="ot", bufs=2)
                    nc.vector.tensor_scalar_mul(outt[:], cx[:], recip[:])
                    nc.sync.dma_start(out_t[qi], outt[:])

                prev = scores(0)
                for qi in range(1, QO):
                    cur = scores(qi)
                    context(qi - 1, *prev)
                    prev = cur
                context(QO - 1, *prev)

                ap.release()
                psa.release()

    nc.compile()
    return nc


def make_in_maps(inputs):
    """Shard FULL inputs into per-core input maps (host-side, untimed)."""
    query = np.asarray(inputs["query"], dtype=np.float32)
    value = np.asarray(inputs["value"], dtype=np.float32)
    Wq = np.asarray(inputs["Wq"], dtype=np.float32)
    Wv = np.asarray(inputs["Wv"], dtype=np.float32)
    bqv = np.asarray(inputs["bq"], dtype=np.float32)
    bvv = np.asarray(inputs["bv"], dtype=np.float32)

    q16 = query.astype(np.float16)
    v16 = value.astype(np.float16)
    # weight pre-tiling (pure layout): [H, A] -> [128, H//128, A]
    wq_t = np.ascontiguousarray(
        Wq.reshape(KO, P, A).transpose(1, 0, 2).astype(np.float16)
    )
    wv_t = np.ascontiguousarray(
        Wv.reshape(KO, P, A).transpose(1, 0, 2).astype(np.float16)
    )
    bq_t = np.ascontiguousarray(bqv.reshape(AO, P).T)
    bv_t = np.ascontiguousarray(bvv.reshape(AO, P).T)

    in_maps = []
    for c in range(N_CORES):
        b, h = c // 2, c % 2
        in_maps.append(
            {
                "xq": np.ascontiguousarray(q16[b, h * SQ : (h + 1) * SQ, :]),
                "xv": np.ascontiguousarray(v16[b]),
                "wq": wq_t,
                "wv": wv_t,
                "bq": bq_t,
                "bv": bv_t,
            }
        )
    return in_maps


_NC_CACHE = {}


def _get_nc():
    if "nc" not in _NC_CACHE:
        _NC_CACHE["nc"] = build()
    return _NC_CACHE["nc"]


def kernel(**inputs):
    nc = _get_nc()
    in_maps = make_in_maps(inputs)
    res = run_bass_kernel_spmd(nc, in_maps, core_ids=list(range(N_CORES)))
    out = np.empty((B, S, A), np.float32)
    for c in range(N_CORES):
        b, h = c // 2, c % 2
        out[b, h * SQ : (h + 1) * SQ, :] = res.results[c]["out"]
    return out
